# revision 2
# baseline (speedup 1.0000x reference)
"""Multi-head self-attention (N=4, T=2048, D=1024, H=16) on 8 TRN2 NeuronCores.

Sharding: core c -> (batch n = c//2, head-group g = c%2 of 8 heads).
Each core projects its batch with its 512-row slices of Wq/Wk/Wv, runs
attention for its 8 heads, AllGathers the per-pair context (bf16, in two
halves so transfer overlaps attention), and computes its 512 output
columns with its slice of Wo.

Query compaction: the reference mask is query-aligned; a masked query's
attention row is exactly uniform (ctx = mean(V)), and attention is
permutation-equivariant over tokens.  The host permutes each batch so
unmasked queries come first, the device processes only the first
U >= max_n(U_n)+1 query columns (U a multiple of 128), and the host
replicates the first masked row (which the device computes as uniform
via its zeroed Q column) into all remaining masked positions.  Keys and
values still cover all T tokens.

Kernel structure per core:
  - X^T via PE transpose; projections emit feature-major Q^T/K^T (f32r)
    and token-major V (bf16) with a ones column per head so the softmax
    denominator falls out of the ctx matmul for free.
  - Masked query columns of Q^T are zeroed: their scores are all equal,
    so unnormalized softmax gives exactly the uniform attention the
    reference's -1e20 masked_fill produces.
  - S^T = K^T.T @ Q^T per (head, query-chunk); exp on ScalarE PSUM->SBUF
    in bf16; ctx^T (+Z row) = [V|1].T @ P^T; normalize with 1/Z.
  - ctx matmuls run one unit behind S/exp so ScalarE never starves.
"""

from contextlib import ExitStack

import numpy as np

import concourse.bass as bass
import concourse.mybir as mybir
import concourse.tile as tile
from concourse import bacc
from concourse.bass_utils import run_bass_kernel_spmd
from concourse.masks import make_identity

N, T, D, H, DH = 4, 2048, 1024, 16, 64
N_CORES = 8
G = 512            # per-core projection width (8 heads x 64)
HPC = 8            # heads per core
SCALE = 1.0 / 8.0  # 1/sqrt(DH)

f32 = mybir.dt.float32
f32r = mybir.dt.float32r
bf16 = mybir.dt.bfloat16
i32 = mybir.dt.int32

COMPUTE_DT = "f32r"  # {"f32r", "bf16"} dtype for projection/S matmul operands

# global din-block order produced by the two half-AllGathers:
# half 0 carries heads 0-3 (blocks 0,1) + peer heads 8-11 (blocks 4,5)
CC_PERM = [[0, 1, 4, 5], [2, 3, 6, 7]]


def _chunks(total, width):
    """[(offset, width)] covering [0, total) in pieces of at most width."""
    out = []
    t0 = 0
    while t0 < total:
        w = min(width, total - t0)
        out.append((t0, w))
        t0 += w
    return out


def build_nc(compute_dt: str = COMPUTE_DT, single_core: bool = False,
             reps: int = 0, u: int = T) -> bacc.Bacc:
    # float32r tiles: PE runs 1 cycle/row (vs 4 for fp32); producers
    # (DVE/ACT evicts) round to f32r precision on write.
    cdt = f32r if compute_dt == "f32r" else bf16
    assert u % 128 == 0 and 128 <= u <= T

    nc = bacc.Bacc(
        "TRN2", target_bir_lowering=False, debug=False, num_devices=N_CORES
    )
    x_d = nc.dram_tensor("query", [T, D], f32, kind="ExternalInput").ap()
    m_d = nc.dram_tensor("mask", [T], i32, kind="ExternalInput").ap()
    wq_d = nc.dram_tensor("Wq", [G, D], f32, kind="ExternalInput").ap()
    wk_d = nc.dram_tensor("Wk", [G, D], f32, kind="ExternalInput").ap()
    wv_d = nc.dram_tensor("Wv", [G, D], f32, kind="ExternalInput").ap()
    wo_d = nc.dram_tensor("Wo", [G, D], f32, kind="ExternalInput").ap()
    bq_d = nc.dram_tensor("bq", [G], f32, kind="ExternalInput").ap()
    bk_d = nc.dram_tensor("bk", [G], f32, kind="ExternalInput").ap()
    bv_d = nc.dram_tensor("bv", [G], f32, kind="ExternalInput").ap()
    bo_d = nc.dram_tensor("bo", [G], f32, kind="ExternalInput").ap()
    out_d = nc.dram_tensor("out", [u, G], f32, kind="ExternalOutput").ap()

    TB = T // 128   # 16 key token blocks
    DB = D // 128   # 8 feature blocks
    GB = G // 128   # 4 projected blocks
    UB = u // 128   # query token blocks

    with tile.TileContext(nc) as tc, ExitStack() as outer_ctx:
        if reps:
            outer_ctx.enter_context(tc.For_i(0, reps, 1))
        ctx = outer_ctx.enter_context(ExitStack())
        const = ctx.enter_context(tc.tile_pool(name="const", bufs=1))
        identity = const.tile([128, 128], f32)
        make_identity(nc, identity)
        bqk = const.tile([128, 2 * GB], f32, tag="bqk")
        bq_c, bk_c = bqk[:, 0:GB], bqk[:, GB:2 * GB]
        bvo = const.tile([128, 2 * G], f32, tag="bvo")
        bvb, bob = bvo[:, 0:G], bvo[:, G:2 * G]

        qpool = ctx.enter_context(tc.tile_pool(name="qpool", bufs=1))
        q_t = [qpool.tile([128, u], cdt, tag=f"q{i}", name=f"q{i}")
               for i in range(GB)]
        k_t = [qpool.tile([128, T], cdt, tag=f"k{i}", name=f"k{i}")
               for i in range(GB)]
        v_all = qpool.tile([128, TB * HPC * 65], bf16, tag="v_all")
        v_t = [v_all[:, i * HPC * 65:(i + 1) * HPC * 65] for i in range(TB)]

        dram = ctx.enter_context(tc.tile_pool(name="dram", bufs=1, space="DRAM"))
        cc_in = dram.tile([G, u], bf16)
        cc_out = [dram.tile([G, u], bf16, name=f"cc_out{c}", tag=f"cc_out{c}")
                  for c in range(2)]

        maskp = ctx.enter_context(ExitStack())
        mkpool = maskp.enter_context(tc.tile_pool(name="maskp", bufs=1))
        maskb = mkpool.tile([128, u], f32, tag="maskb")

        # ---- mask + biases ----
        with tc.tile_pool(name="mload", bufs=1) as mp:
            m_i = mp.tile([1, T], i32)
            nc.sync.dma_start(m_i[:], m_d[None, :])
            m_f = mp.tile([1, T], f32)
            nc.vector.tensor_copy(m_f[:], m_i[:])
            nc.gpsimd.partition_broadcast(maskb[:], m_f[:, 0:u])
            nc.sync.dma_start(bq_c[:], bq_d.rearrange("(j p) -> p j", p=128))
            nc.sync.dma_start(bk_c[:], bk_d.rearrange("(j p) -> p j", p=128))
            bv_r = mp.tile([1, G], f32, tag="bvr")
            nc.sync.dma_start(bv_r[:], bv_d[None, :])
            nc.gpsimd.partition_broadcast(bvb[:], bv_r[:])
            bo_r = mp.tile([1, G], f32, tag="bor")
            nc.sync.dma_start(bo_r[:], bo_d[None, :])
            nc.gpsimd.partition_broadcast(bob[:], bo_r[:])

        # ---- phase 1: X^T, W^T, projections ----
        att = ctx.enter_context(ExitStack())

        def open_att_pools():
            slabp = att.enter_context(tc.tile_pool(name="slab", bufs=2))
            zp = att.enter_context(tc.tile_pool(name="zbuf", bufs=2))
            csp = att.enter_context(tc.tile_pool(name="cstage", bufs=3))
            spp = att.enter_context(
                tc.tile_pool(name="spsum", bufs=2, space="PSUM"))
            return slabp, zp, csp, spp

        p1 = ctx.enter_context(ExitStack())
        if True:
            xtp = p1.enter_context(tc.tile_pool(name="xt", bufs=1))
            sp = p1.enter_context(tc.tile_pool(name="stage", bufs=3))
            wtp = p1.enter_context(tc.tile_pool(name="wt", bufs=16))
            pp = p1.enter_context(
                tc.tile_pool(name="pp", bufs=4, space="PSUM"))
            xt = [xtp.tile([128, T], cdt, tag=f"xt{d}", name=f"xt{d}")
                  for d in range(DB)]
            for i in range(TB):
                xs = sp.tile([128, D], f32, tag="stage", name="stage")
                if i < 2:
                    # split the pipe-filling loads so the first transpose
                    # can start as early as possible
                    for ii in range(4):
                        nc.sync.dma_start(
                            xs[ii * 32:(ii + 1) * 32, :],
                            x_d[i * 128 + ii * 32:i * 128 + (ii + 1) * 32, :],
                        )
                else:
                    nc.sync.dma_start(xs[:], x_d[i * 128:(i + 1) * 128, :])
                for d in range(DB):
                    ps = pp.tile([128, 512], f32, tag="pp", name="pp")
                    nc.tensor.transpose(
                        ps[:, 0:128], xs[:, d * 128:(d + 1) * 128], identity[:]
                    )
                    nc.any.tensor_copy(
                        xt[d][:, i * 128:(i + 1) * 128], ps[:, 0:128]
                    )

            def load_wT(w_dram):
                tiles = [wtp.tile([128, G], cdt, tag="wt", name="wt")
                         for _ in range(DB)]
                for r in range(GB):
                    ws = sp.tile([128, D], f32, tag="stage", name="stage")
                    nc.sync.dma_start(ws[:], w_dram[r * 128:(r + 1) * 128, :])
                    for d in range(DB):
                        ps = pp.tile([128, 512], f32, tag="pp", name="pp")
                        nc.tensor.transpose(
                            ps[:, 0:128], ws[:, d * 128:(d + 1) * 128],
                            identity[:]
                        )
                        nc.any.tensor_copy(
                            tiles[d][:, r * 128:(r + 1) * 128], ps[:, 0:128]
                        )
                return tiles

            # Q^T with masked-query columns zeroed (-> uniform softmax rows,
            # matching the reference's -1e20 masked_fill exactly)
            wqT = load_wT(wq_d)
            for b in range(GB):
                for (t0, w) in _chunks(u, 512):
                    ps = pp.tile([128, 512], f32, tag="pp", name="pp")
                    for d in range(DB):
                        nc.tensor.matmul(
                            ps[:, 0:w],
                            wqT[d][:, b * 128:(b + 1) * 128],
                            xt[d][:, t0:t0 + w],
                            start=(d == 0),
                            stop=(d == DB - 1),
                        )
                    nc.vector.scalar_tensor_tensor(
                        q_t[b][:, t0:t0 + w],
                        ps[:, 0:w],
                        bq_c[:, b:b + 1],
                        maskb[:, t0:t0 + w],
                        op0=mybir.AluOpType.add,
                        op1=mybir.AluOpType.mult,
                    )
            wkT = load_wT(wk_d)
            for b in range(GB):
                for tch in range(4):
                    ps = pp.tile([128, 512], f32, tag="pp", name="pp")
                    for d in range(DB):
                        nc.tensor.matmul(
                            ps[:],
                            wkT[d][:, b * 128:(b + 1) * 128],
                            xt[d][:, tch * 512:(tch + 1) * 512],
                            start=(d == 0),
                            stop=(d == DB - 1),
                        )
                    nc.vector.tensor_scalar_add(
                        k_t[b][:, tch * 512:(tch + 1) * 512], ps[:],
                        bk_c[:, b:b + 1]
                    )

            def emit_v_proj():
                # V token-major [t, dout] with a ones column per head
                wvT = load_wT(wv_d)
                for i in range(TB):
                    nc.gpsimd.memset(v_t[i][:], 1.0)
                    ps = pp.tile([128, 512], f32, tag="pp", name="pp")
                    for d in range(DB):
                        nc.tensor.matmul(
                            ps[:],
                            xt[d][:, i * 128:(i + 1) * 128],
                            wvT[d][:],
                            start=(d == 0),
                            stop=(d == DB - 1),
                        )
                    for h in range(HPC):
                        nc.vector.tensor_tensor(
                            v_t[i][:, h * 65:h * 65 + 64],
                            ps[:, h * 64:(h + 1) * 64],
                            bvb[:, h * 64:(h + 1) * 64],
                            op=mybir.AluOpType.add,
                        )

            emit_v_proj()
            p1.close()
            maskp.close()
            att_pools = open_att_pools()

        # ---- phase 2: attention; ctx runs 1-2 units behind S/exp ----
        qunits = _chunks(u, 1024)
        units = [(h, t0, w) for h in range(HPC) for (t0, w) in qunits]
        if True:
            slabp, zp, csp, spp = att_pools
            slabs = {}

            def emit_s_exp(unit):
                h, t0, w = unit
                qk, hb = h // 2, (h % 2) * 64
                slab = slabp.tile([128, 16 * 1024], bf16, tag="slab",
                                  name="slab")
                slabs[unit] = slab
                for j in range(TB):
                    sps = spp.tile([128, 1024], f32, tag="sp", name="sp")
                    for (q0, qw) in _chunks(w, 512):
                        nc.tensor.matmul(
                            sps[:, q0:q0 + qw],
                            k_t[qk][hb:hb + 64, j * 128:(j + 1) * 128],
                            q_t[qk][hb:hb + 64, t0 + q0:t0 + q0 + qw],
                            start=True,
                            stop=True,
                        )
                    nc.scalar.activation(
                        slab[:, j * 1024:j * 1024 + w],
                        sps[:, 0:w],
                        mybir.ActivationFunctionType.Exp,
                        scale=SCALE,
                    )

            late = ExitStack()
            cpp = late.enter_context(
                tc.tile_pool(name="cpsum", bufs=2, space="PSUM"))
            woTp = late.enter_context(tc.tile_pool(name="wot", bufs=8))
            cf0p = late.enter_context(tc.tile_pool(name="cf0", bufs=1))
            woT = [woTp.tile([128, G], bf16, tag="wot", name="wot")
                   for _ in range(DB)]
            cf0 = [cf0p.tile([128, u], bf16, tag=f"cf0{j}", name=f"cf0{j}")
                   for j in range(GB)]

            def emit_collective(c, cf_tiles):
                if single_core:
                    nc.sync.dma_start(
                        cc_out[c][0:256, :], cc_in[c * 256:(c + 1) * 256, :]
                    )
                    nc.sync.dma_start(
                        cc_out[c][256:512, :], cc_in[c * 256:(c + 1) * 256, :]
                    )
                else:
                    nc.gpsimd.collective_compute(
                        "AllGather",
                        mybir.AluOpType.bypass,
                        replica_groups=[[0, 1], [2, 3], [4, 5], [6, 7]],
                        ins=[cc_in[c * 256:(c + 1) * 256, :].opt()],
                        outs=[cc_out[c][:].opt()],
                    )
                for j in range(GB):
                    nc.sync.dma_start(
                        cf_tiles[j][:], cc_out[c][j * 128:(j + 1) * 128, :]
                    )

            def emit_woT_prep():
                for r in range(GB):
                    ws = csp.tile([128, D], f32, tag="cst", name="wos")
                    nc.sync.dma_start(ws[:], wo_d[r * 128:(r + 1) * 128, :])
                    for d in range(DB):
                        ps = cpp.tile([128, 512], f32, tag="cp", name="cp")
                        nc.tensor.transpose(
                            ps[:, 0:128], ws[:, d * 128:(d + 1) * 128],
                            identity[:]
                        )
                        nc.vector.tensor_copy(
                            woT[d][:, r * 128:(r + 1) * 128], ps[:, 0:128]
                        )

            def emit_ctx(unit):
                h, t0, w = unit
                slab = slabs.pop(unit)
                cps = cpp.tile([65, 1024], f32, tag="cp", name="cp")
                for (q0, qw) in _chunks(w, 512):
                    for j in range(TB):
                        nc.tensor.matmul(
                            cps[:, q0:q0 + qw],
                            v_t[j][:, h * 65:h * 65 + 65],
                            slab[:, j * 1024 + q0:j * 1024 + q0 + qw],
                            start=(j == 0),
                            stop=(j == TB - 1),
                        )
                # row 64 holds Z = sum_k exp; scale rows 0..63 by 1/Z
                zrow = zp.tile([128, 1024], f32, tag="z", name="z")
                nc.vector.tensor_copy(zrow[64:65, 0:w], cps[64:65, 0:w])
                nc.sync.dma_start(zrow[0:1, 0:w], zrow[64:65, 0:w])
                nc.vector.reciprocal(zrow[0:1, 0:w], zrow[0:1, 0:w])
                bct = zp.tile([64, 1024], f32, tag="bc", name="bc", bufs=1)
                nc.gpsimd.partition_broadcast(bct[:, 0:w], zrow[0:1, 0:w])
                cst = csp.tile([64, 1024], bf16, tag="cst", name="cst")
                nc.vector.tensor_tensor(
                    cst[:, 0:w], cps[0:64, 0:w], bct[:, 0:w],
                    op=mybir.AluOpType.mult
                )
                nc.sync.dma_start(
                    cc_in[h * 64:(h + 1) * 64, t0:t0 + w], cst[:, 0:w]
                )

            for idx, unit in enumerate(units):
                emit_s_exp(unit)
                if idx > 0:
                    emit_ctx(units[idx - 1])
                if idx == min(4, len(units) - 1):
                    emit_woT_prep()
                if idx == len(units) // 2 + 1:
                    emit_collective(0, cf0)
            emit_ctx(units[-1])

            # ---- phase 3: second gather half + output projection ----
            # cf1 reuses a slab slot (attention has drained by now)
            if True:
                cf1_all = slabp.tile([128, 16 * 1024], bf16, tag="slab",
                                     name="cf1")
                cf1 = [cf1_all[:, j * u:(j + 1) * u] for j in range(GB)]
                emit_collective(1, cf1)
                cf = [cf0, cf1]
                for i in range(UB):
                    ps = cpp.tile([128, 512], f32, tag="cp", name="cp")
                    first = True
                    for c in range(2):
                        for j in range(GB):
                            nc.tensor.matmul(
                                ps[:],
                                cf[c][j][:, i * 128:(i + 1) * 128],
                                woT[CC_PERM[c][j]][:],
                                start=first,
                                stop=(c == 1 and j == GB - 1),
                            )
                            first = False
                    os_ = csp.tile([128, G], f32, tag="ostage", name="ostage")
                    nc.vector.tensor_tensor(os_[:], ps[:], bob[:],
                                            op=mybir.AluOpType.add)
                    nc.sync.dma_start(out_d[i * 128:(i + 1) * 128, :], os_[:])
            late.close()

    nc.compile()
    return nc


def _prep_permutation(mask):
    """Per-batch permutation putting unmasked queries first, and the padded
    processed-query count U (multiple of 128, > U_n for every batch with
    masked queries)."""
    perms, uns = [], []
    for n in range(N):
        m = np.asarray(mask[n])
        unm = np.flatnonzero(m != 0)
        msk = np.flatnonzero(m == 0)
        perms.append(np.concatenate([unm, msk]).astype(np.int64))
        uns.append(int(len(unm)))
    umax = max(uns)
    u = min(T, ((umax + 1 + 127) // 128) * 128)
    return perms, uns, u


def shard_inputs(query, mask, Wq, bq, Wk, bk, Wv, bv, Wo, bo):
    perms, uns, u = _prep_permutation(mask)
    meta = {"perms": perms, "uns": uns, "u": u}
    in_maps = []
    for c in range(N_CORES):
        n, g = c // 2, c % 2
        sl = slice(g * G, (g + 1) * G)
        in_maps.append(
            {
                "query": np.ascontiguousarray(
                    np.asarray(query[n])[perms[n]], dtype=np.float32),
                "mask": np.ascontiguousarray(
                    np.asarray(mask[n])[perms[n]], dtype=np.int32),
                "Wq": np.ascontiguousarray(Wq[sl], dtype=np.float32),
                "Wk": np.ascontiguousarray(Wk[sl], dtype=np.float32),
                "Wv": np.ascontiguousarray(Wv[sl], dtype=np.float32),
                "Wo": np.ascontiguousarray(Wo[sl], dtype=np.float32),
                "bq": np.ascontiguousarray(bq[sl], dtype=np.float32),
                "bk": np.ascontiguousarray(bk[sl], dtype=np.float32),
                "bv": np.ascontiguousarray(bv[sl], dtype=np.float32),
                "bo": np.ascontiguousarray(bo[sl], dtype=np.float32),
            }
        )
    return in_maps, meta


def gather_outputs(results, meta):
    perms, uns, u = meta["perms"], meta["uns"], meta["u"]
    out = np.empty((N, T, D), np.float32)
    for n in range(N):
        dev = np.empty((u, D), np.float32)
        for g in range(2):
            dev[:, g * G:(g + 1) * G] = results[2 * n + g]["out"]
        perm = perms[n]
        out[n][perm[:u]] = dev
        if u < T:
            # all masked queries share the uniform-attention output row;
            # uns[n] < u is guaranteed, and row uns[n] is masked
            out[n][perm[u:]] = dev[uns[n]]
    return out


def kernel(query, mask, Wq, bq, Wk, bk, Wv, bv, Wo, bo):
    in_maps, meta = shard_inputs(query, mask, Wq, bq, Wk, bk, Wv, bv, Wo, bo)
    nc = build_nc(u=meta["u"])
    res = run_bass_kernel_spmd(nc, in_maps, list(range(N_CORES)))
    return gather_outputs(res.results, meta)


# revision 8
# speedup vs baseline: 2.1614x; 2.1614x over previous
"""Multi-head self-attention (N=4, T=2048, D=1024, H=16) on 8 TRN2 NeuronCores.

Sharding: core c -> (batch n = c//2, head-group g = c%2 of 8 heads).
Each core projects its batch with its 512-row slices of Wq/Wk/Wv, runs
attention for its 8 heads, AllGathers the per-pair context (bf16, in two
halves so transfer overlaps attention), and computes its 512 output
columns with its slice of Wo.

Query compaction: the reference mask is query-aligned; a masked query's
attention row is exactly uniform (ctx = mean(V)), and attention is
permutation-equivariant over tokens.  The host permutes each batch so
unmasked queries come first, the device processes only the first
U >= max_n(U_n)+1 query columns (U a multiple of 128), and the host
replicates the first masked row (which the device computes as uniform
via its zeroed Q column) into all remaining masked positions.  Keys and
values still cover all T tokens.

Host-side layout: the host feeds X^T and Wq^T/Wk^T/Wv^T/Wo^T as bf16,
so the kernel needs no PE transposes and no staging/eviction traffic —
feature-major operands DMA straight into SBUF.

Kernel structure per core:
  - Masked query columns of Q^T are zeroed: their scores are all equal,
    so unnormalized softmax gives exactly the uniform attention the
    reference's -1e20 masked_fill produces.
  - S^T per head PAIR: heads 2p/2p+1 live on partitions 0-63/64-127 of
    q_t[p]/k_t[p], so their K=64 matmuls auto-derive row groups (0,*)
    and (64,*) and run concurrently in the PE array (row tiling).
  - exp on ScalarE PSUM->SBUF bf16 covers both heads in one N=2w
    activation; ctx^T (+Z row) = [V|1].T @ P^T per head; normalize with
    1/Z via reciprocal_approx_fast + gpsimd broadcast.
  - V projection is emitted under the first unit's exp window; ctx runs
    one unit behind S/exp so ScalarE never starves.
"""

from contextlib import ExitStack

import numpy as np

import concourse.bass as bass
import concourse.mybir as mybir
import concourse.tile as tile
from concourse import bacc
from concourse.bass_utils import run_bass_kernel_spmd

N, T, D, H, DH = 4, 2048, 1024, 16, 64
N_CORES = 8
G = 512            # per-core projection width (8 heads x 64)
HPC = 8            # heads per core
SCALE = 1.0 / 8.0  # 1/sqrt(DH)

f32 = mybir.dt.float32
bf16 = mybir.dt.bfloat16
i32 = mybir.dt.int32

COMPUTE_DT = "bf16"  # operands are bf16 (host-converted); kept for test.py

# global din-block order produced by the two half-AllGathers:
# half 0 carries heads 0-3 (blocks 0,1) + peer heads 8-11 (blocks 4,5)
CC_PERM = [[0, 1, 4, 5], [2, 3, 6, 7]]


def _chunks(total, width):
    """[(offset, width)] covering [0, total) in pieces of at most width."""
    out = []
    t0 = 0
    while t0 < total:
        w = min(width, total - t0)
        out.append((t0, w))
        t0 += w
    return out


def build_nc(compute_dt: str = COMPUTE_DT, single_core: bool = False,
             reps: int = 0, u: int = T) -> bacc.Bacc:
    assert u % 128 == 0 and 128 <= u <= T

    nc = bacc.Bacc(
        "TRN2", target_bir_lowering=False, debug=False, num_devices=N_CORES
    )
    xT_d = nc.dram_tensor("xT", [D, T], bf16, kind="ExternalInput").ap()
    m_d = nc.dram_tensor("mask", [T], i32, kind="ExternalInput").ap()
    wqT_d = nc.dram_tensor("wqT", [D, G], bf16, kind="ExternalInput").ap()
    wkT_d = nc.dram_tensor("wkT", [D, G], bf16, kind="ExternalInput").ap()
    wvT_d = nc.dram_tensor("wvT", [D, G], bf16, kind="ExternalInput").ap()
    woT_d = nc.dram_tensor("woT", [D, G], bf16, kind="ExternalInput").ap()
    bq_d = nc.dram_tensor("bq", [G], f32, kind="ExternalInput").ap()
    bk_d = nc.dram_tensor("bk", [G], f32, kind="ExternalInput").ap()
    bv_d = nc.dram_tensor("bv", [G], f32, kind="ExternalInput").ap()
    bo_d = nc.dram_tensor("bo", [G], f32, kind="ExternalInput").ap()
    out_d = nc.dram_tensor("out", [u, G], f32, kind="ExternalOutput").ap()

    TB = T // 128   # 16 key token blocks
    DB = D // 128   # 8 feature blocks
    GB = G // 128   # 4 projected blocks
    UB = u // 128   # query token blocks
    qchunks = _chunks(u, 512)
    nch = len(qchunks)

    with tile.TileContext(nc) as tc, ExitStack() as outer_ctx:
        if reps:
            outer_ctx.enter_context(tc.For_i(0, reps, 1))
        ctx = outer_ctx.enter_context(ExitStack())
        const = ctx.enter_context(tc.tile_pool(name="const", bufs=1))
        bqk = const.tile([128, 2 * GB], f32, tag="bqk")
        bq_c, bk_c = bqk[:, 0:GB], bqk[:, GB:2 * GB]
        bvo = const.tile([128, 2 * G], f32, tag="bvo")
        bvb, bob = bvo[:, 0:G], bvo[:, G:2 * G]

        qpool = ctx.enter_context(tc.tile_pool(name="qpool", bufs=1))
        q_t = [qpool.tile([128, u], bf16, tag=f"q{i}", name=f"q{i}")
               for i in range(GB)]
        k_t = [qpool.tile([128, T], bf16, tag=f"k{i}", name=f"k{i}")
               for i in range(GB)]
        v_all = qpool.tile([128, TB * HPC * 65], bf16, tag="v_all")
        v_t = [v_all[:, i * HPC * 65:(i + 1) * HPC * 65] for i in range(TB)]

        dram = ctx.enter_context(tc.tile_pool(name="dram", bufs=1, space="DRAM"))
        cc_in = dram.tile([G, u], bf16)
        cc_out = [dram.tile([G, u], bf16, name=f"cc_out{c}", tag=f"cc_out{c}")
                  for c in range(2)]

        mkpool = ctx.enter_context(tc.tile_pool(name="maskp", bufs=1))
        maskb = mkpool.tile([128, u], f32, tag="maskb")

        # ---- mask + biases ----
        with tc.tile_pool(name="mload", bufs=1) as mp:
            m_i = mp.tile([1, T], i32)
            nc.sync.dma_start(m_i[:], m_d[None, :])
            m_f = mp.tile([1, T], f32)
            nc.vector.tensor_copy(m_f[:], m_i[:])
            nc.gpsimd.partition_broadcast(maskb[:], m_f[:, 0:u])
            nc.sync.dma_start(bq_c[:], bq_d.rearrange("(j p) -> p j", p=128))
            nc.sync.dma_start(bk_c[:], bk_d.rearrange("(j p) -> p j", p=128))
            bv_r = mp.tile([1, G], f32, tag="bvr")
            nc.sync.dma_start(bv_r[:], bv_d[None, :])
            nc.gpsimd.partition_broadcast(bvb[:], bv_r[:])
            bo_r = mp.tile([1, G], f32, tag="bor")
            nc.sync.dma_start(bo_r[:], bo_d[None, :])
            nc.gpsimd.partition_broadcast(bob[:], bo_r[:])

        # attention pools open FIRST so phase-1 pools can release while
        # attention is running (pool stack releases LIFO)
        att = ctx.enter_context(ExitStack())
        slabp = att.enter_context(tc.tile_pool(name="slab", bufs=2))
        zp = att.enter_context(tc.tile_pool(name="zbuf", bufs=2))
        csp = att.enter_context(tc.tile_pool(name="cstage", bufs=3))
        spp = att.enter_context(tc.tile_pool(name="spsum", bufs=2,
                                             space="PSUM"))

        # ---- phase 1: direct loads + K/Q projections ----
        p1 = ctx.enter_context(ExitStack())
        xtp = p1.enter_context(tc.tile_pool(name="xt", bufs=1))
        wtp = p1.enter_context(tc.tile_pool(name="wt", bufs=24))
        pp = p1.enter_context(tc.tile_pool(name="pp", bufs=4, space="PSUM"))

        xt = [xtp.tile([128, T], bf16, tag=f"xt{d}", name=f"xt{d}")
              for d in range(DB)]
        for tch in range(4):
            for d in range(DB):
                nc.sync.dma_start(
                    xt[d][:, tch * 512:(tch + 1) * 512],
                    xT_d[d * 128:(d + 1) * 128, tch * 512:(tch + 1) * 512],
                )

        def load_wT(w_dram):
            tiles = [wtp.tile([128, G], bf16, tag="wt", name="wt")
                     for _ in range(DB)]
            for d in range(DB):
                nc.sync.dma_start(tiles[d][:],
                                  w_dram[d * 128:(d + 1) * 128, :])
            return tiles

        wkT_t = load_wT(wkT_d)
        wqT_t = load_wT(wqT_d)
        wvT_t = load_wT(wvT_d)

        # K^T feature-major [dout, token]
        for b in range(GB):
            for tch in range(4):
                ps = pp.tile([128, 512], f32, tag="pp", name="pp")
                for d in range(DB):
                    nc.tensor.matmul(
                        ps[:],
                        wkT_t[d][:, b * 128:(b + 1) * 128],
                        xt[d][:, tch * 512:(tch + 1) * 512],
                        start=(d == 0),
                        stop=(d == DB - 1),
                    )
                nc.vector.tensor_scalar_add(
                    k_t[b][:, tch * 512:(tch + 1) * 512], ps[:],
                    bk_c[:, b:b + 1]
                )
        # Q^T with masked-query columns zeroed (-> uniform softmax rows,
        # matching the reference's -1e20 masked_fill exactly)
        for b in range(GB):
            for (t0, w) in qchunks:
                ps = pp.tile([128, 512], f32, tag="pp", name="pp")
                for d in range(DB):
                    nc.tensor.matmul(
                        ps[:, 0:w],
                        wqT_t[d][:, b * 128:(b + 1) * 128],
                        xt[d][:, t0:t0 + w],
                        start=(d == 0),
                        stop=(d == DB - 1),
                    )
                nc.vector.scalar_tensor_tensor(
                    q_t[b][:, t0:t0 + w],
                    ps[:, 0:w],
                    bq_c[:, b:b + 1],
                    maskb[:, t0:t0 + w],
                    op0=mybir.AluOpType.add,
                    op1=mybir.AluOpType.mult,
                )

        def emit_v_proj():
            # V token-major [t, dout] with a ones column per head
            nc.gpsimd.memset(v_all[:], 1.0)
            for i in range(TB):
                ps = pp.tile([128, 512], f32, tag="pp", name="pp")
                for d in range(DB):
                    nc.tensor.matmul(
                        ps[:],
                        xt[d][:, i * 128:(i + 1) * 128],
                        wvT_t[d][:],
                        start=(d == 0),
                        stop=(d == DB - 1),
                    )
                for h in range(HPC):
                    nc.vector.tensor_tensor(
                        v_t[i][:, h * 65:h * 65 + 64],
                        ps[:, h * 64:(h + 1) * 64],
                        bvb[:, h * 64:(h + 1) * 64],
                        op=mybir.AluOpType.add,
                    )

        # ---- phase 2: attention per head pair; ctx one unit behind ----
        units = [(p, t0, w) for p in range(GB) for (t0, w) in qchunks]
        slabs = {}

        def emit_s_exp(unit):
            # the two heads' K=64 matmuls run CONCURRENTLY (row groups 0/64),
            # so head B always gets its own PSUM bank (offset 512) — two
            # concurrent drains into one bank are a hard PSUM fault
            p, t0, w = unit
            slab = slabp.tile([128, TB * 1024], bf16, tag="slab", name="slab")
            slabs[unit] = slab
            for j in range(TB):
                sps = spp.tile([128, 1024], f32, tag="sp", name="sp")
                nc.tensor.matmul(
                    sps[:, 0:w],
                    k_t[p][0:64, j * 128:(j + 1) * 128],
                    q_t[p][0:64, t0:t0 + w],
                    start=True, stop=True,
                )
                nc.tensor.matmul(
                    sps[:, 512:512 + w],
                    k_t[p][64:128, j * 128:(j + 1) * 128],
                    q_t[p][64:128, t0:t0 + w],
                    start=True, stop=True,
                )
                if w == 512:
                    nc.scalar.activation(
                        slab[:, j * 1024:j * 1024 + 1024],
                        sps[:],
                        mybir.ActivationFunctionType.Exp,
                        scale=SCALE,
                    )
                else:
                    for hh in range(2):
                        nc.scalar.activation(
                            slab[:, j * 1024 + hh * 512:
                                  j * 1024 + hh * 512 + w],
                            sps[:, hh * 512:hh * 512 + w],
                            mybir.ActivationFunctionType.Exp,
                            scale=SCALE,
                        )

        late = ExitStack()
        woT = []
        cf0 = []
        cpp = None

        def open_late_pools():
            # called after p1.close() so cpsum can take the freed PSUM banks
            nonlocal cpp
            cpp = late.enter_context(
                tc.tile_pool(name="cpsum", bufs=4, space="PSUM"))
            woTp = late.enter_context(tc.tile_pool(name="wot", bufs=8))
            cf0p = late.enter_context(tc.tile_pool(name="cf0", bufs=1))
            woT.extend(woTp.tile([128, G], bf16, tag="wot", name="wot")
                       for _ in range(DB))
            for d in range(DB):
                nc.sync.dma_start(woT[d][:], woT_d[d * 128:(d + 1) * 128, :])
            cf0.extend(cf0p.tile([128, u], bf16, tag=f"cf0{j}",
                                 name=f"cf0{j}")
                       for j in range(GB))

        def emit_collective(c, cf_tiles):
            if single_core:
                nc.sync.dma_start(
                    cc_out[c][0:256, :], cc_in[c * 256:(c + 1) * 256, :]
                )
                nc.sync.dma_start(
                    cc_out[c][256:512, :], cc_in[c * 256:(c + 1) * 256, :]
                )
            else:
                nc.gpsimd.collective_compute(
                    "AllGather",
                    mybir.AluOpType.bypass,
                    replica_groups=[[0, 1], [2, 3], [4, 5], [6, 7]],
                    ins=[cc_in[c * 256:(c + 1) * 256, :].opt()],
                    outs=[cc_out[c][:].opt()],
                )
            for j in range(GB):
                nc.sync.dma_start(
                    cf_tiles[j][:], cc_out[c][j * 128:(j + 1) * 128, :]
                )

        def emit_post(unit):
            p, t0, w = unit
            b_off = 512
            slab = slabs.pop(unit)
            # ctx^T (+Z row) per head of the pair
            cps = []
            for hh in range(2):
                cp = cpp.tile([65, 512], f32, tag="cp", name="cp")
                cps.append(cp)
                for j in range(TB):
                    nc.tensor.matmul(
                        cp[:, 0:w],
                        v_t[j][:, (2 * p + hh) * 65:(2 * p + hh) * 65 + 65],
                        slab[:, j * 1024 + hh * b_off:
                              j * 1024 + hh * b_off + w],
                        start=(j == 0),
                        stop=(j == TB - 1),
                    )
            # row 64 holds Z = sum_k exp; scale rows 0..63 by 1/Z
            zsb = zp.tile([128, 1024], f32, tag="z", name="z")
            nc.vector.tensor_copy(zsb[64:65, 0:w], cps[0][64:65, 0:w])
            nc.vector.tensor_copy(zsb[64:65, w:2 * w], cps[1][64:65, 0:w])
            nc.sync.dma_start(zsb[0:1, 0:2 * w], zsb[64:65, 0:2 * w])
            nc.vector.reciprocal(zsb[0:1, 0:2 * w], zsb[0:1, 0:2 * w])
            bct = zp.tile([64, 1024], f32, tag="bc", name="bc")
            nc.gpsimd.partition_broadcast(bct[:, 0:2 * w], zsb[0:1, 0:2 * w])
            for hh in range(2):
                cst = csp.tile([64, 512], bf16, tag="cst", name="cst")
                nc.vector.tensor_tensor(
                    cst[:, 0:w], cps[hh][0:64, 0:w],
                    bct[:, hh * w:(hh + 1) * w],
                    op=mybir.AluOpType.mult,
                )
                nc.sync.dma_start(
                    cc_in[p * 128 + hh * 64:p * 128 + hh * 64 + 64,
                          t0:t0 + w],
                    cst[:, 0:w],
                )

        for idx, unit in enumerate(units):
            emit_s_exp(unit)
            if idx == 0:
                # V projection runs under the first unit's exp window
                emit_v_proj()
                p1.close()
                open_late_pools()
            else:
                emit_post(units[idx - 1])
            if idx == min(2 * nch, len(units) - 1):
                emit_collective(0, cf0)
        emit_post(units[-1])

        # ---- phase 3: second gather half + output projection ----
        # cf1 reuses a slab slot (attention has drained by now)
        cf1_all = slabp.tile([128, TB * 1024], bf16, tag="slab", name="cf1")
        cf1 = [cf1_all[:, j * u:(j + 1) * u] for j in range(GB)]
        emit_collective(1, cf1)
        cf = [cf0, cf1]
        for i in range(UB):
            ps = cpp.tile([128, 512], f32, tag="cp", name="cp")
            first = True
            for c in range(2):
                for j in range(GB):
                    nc.tensor.matmul(
                        ps[:],
                        cf[c][j][:, i * 128:(i + 1) * 128],
                        woT[CC_PERM[c][j]][:],
                        start=first,
                        stop=(c == 1 and j == GB - 1),
                    )
                    first = False
            os_ = csp.tile([128, G], f32, tag="ostage", name="ostage")
            nc.vector.tensor_tensor(os_[:], ps[:], bob[:],
                                    op=mybir.AluOpType.add)
            nc.sync.dma_start(out_d[i * 128:(i + 1) * 128, :], os_[:])
        late.close()

    nc.compile()
    return nc


def _prep_permutation(mask):
    """Per-batch permutation putting unmasked queries first, and the padded
    processed-query count U (multiple of 128, > U_n for every batch with
    masked queries)."""
    perms, uns = [], []
    for n in range(N):
        m = np.asarray(mask[n])
        unm = np.flatnonzero(m != 0)
        msk = np.flatnonzero(m == 0)
        perms.append(np.concatenate([unm, msk]).astype(np.int64))
        uns.append(int(len(unm)))
    umax = max(uns)
    u = min(T, ((umax + 1 + 127) // 128) * 128)
    return perms, uns, u


def shard_inputs(query, mask, Wq, bq, Wk, bk, Wv, bv, Wo, bo):
    perms, uns, u = _prep_permutation(mask)
    meta = {"perms": perms, "uns": uns, "u": u}
    nbf = mybir.dt.np(bf16)
    in_maps = []
    for c in range(N_CORES):
        n, g = c // 2, c % 2
        sl = slice(g * G, (g + 1) * G)
        in_maps.append(
            {
                "xT": np.ascontiguousarray(
                    np.asarray(query[n])[perms[n]].T.astype(nbf)),
                "mask": np.ascontiguousarray(
                    np.asarray(mask[n])[perms[n]], dtype=np.int32),
                "wqT": np.ascontiguousarray(Wq[sl].T.astype(nbf)),
                "wkT": np.ascontiguousarray(Wk[sl].T.astype(nbf)),
                "wvT": np.ascontiguousarray(Wv[sl].T.astype(nbf)),
                "woT": np.ascontiguousarray(Wo[sl].T.astype(nbf)),
                "bq": np.ascontiguousarray(bq[sl], dtype=np.float32),
                "bk": np.ascontiguousarray(bk[sl], dtype=np.float32),
                "bv": np.ascontiguousarray(bv[sl], dtype=np.float32),
                "bo": np.ascontiguousarray(bo[sl], dtype=np.float32),
            }
        )
    return in_maps, meta


def gather_outputs(results, meta):
    perms, uns, u = meta["perms"], meta["uns"], meta["u"]
    out = np.empty((N, T, D), np.float32)
    for n in range(N):
        dev = np.empty((u, D), np.float32)
        for g in range(2):
            dev[:, g * G:(g + 1) * G] = results[2 * n + g]["out"]
        perm = perms[n]
        out[n][perm[:u]] = dev
        if u < T:
            # all masked queries share the uniform-attention output row;
            # uns[n] < u is guaranteed, and row uns[n] is masked
            out[n][perm[u:]] = dev[uns[n]]
    return out


def kernel(query, mask, Wq, bq, Wk, bk, Wv, bv, Wo, bo):
    in_maps, meta = shard_inputs(query, mask, Wq, bq, Wk, bk, Wv, bv, Wo, bo)
    nc = build_nc(u=meta["u"])
    res = run_bass_kernel_spmd(nc, in_maps, list(range(N_CORES)))
    return gather_outputs(res.results, meta)


# revision 13
# speedup vs baseline: 2.5192x; 1.1655x over previous
"""Multi-head self-attention (N=4, T=2048, D=1024, H=16) on 8 TRN2 NeuronCores.

Sharding: core c -> (batch n = c//2, head-group g = c%2 of 8 heads).
Each core projects its batch with its 512-row slices of Wq/Wk/Wv, runs
attention for its 8 heads, AllGathers the per-pair context (bf16, in two
halves so transfer overlaps attention), and computes its 512 output
columns with its slice of Wo.

Query compaction: the reference mask is query-aligned; a masked query's
attention row is exactly uniform (ctx = mean(V)), and attention is
permutation-equivariant over tokens.  The host permutes each batch so
unmasked queries come first, the device processes only the first
U >= max_n(U_n)+1 query columns (U a multiple of 128), and the host
replicates the first masked row (which the device computes as uniform
via its zeroed Q column) into all remaining masked positions.  Keys and
values still cover all T tokens.

Host-side layout: the host feeds X^T and Wq^T/Wk^T/Wv^T/Wo^T as bf16,
so the kernel needs no PE transposes and no staging/eviction traffic —
feature-major operands DMA straight into SBUF.

Kernel structure per core:
  - Masked query columns of Q^T are zeroed: their scores are all equal,
    so unnormalized softmax gives exactly the uniform attention the
    reference's -1e20 masked_fill produces.
  - S^T per head PAIR: heads 2p/2p+1 live on partitions 0-63/64-127 of
    q_t[p]/k_t[p], so their K=64 matmuls auto-derive row groups (0,*)
    and (64,*) and run concurrently in the PE array (row tiling).
  - exp on ScalarE PSUM->SBUF bf16 covers both heads in one N=2w
    activation; ctx^T (+Z row) = [V|1].T @ P^T per head; normalize with
    1/Z via reciprocal_approx_fast + gpsimd broadcast.
  - V projection is emitted under the first unit's exp window; ctx runs
    one unit behind S/exp so ScalarE never starves.
"""

from contextlib import ExitStack

import numpy as np

import concourse.bass as bass
import concourse.mybir as mybir
import concourse.tile as tile
from concourse import bacc
from concourse.bass_utils import run_bass_kernel_spmd

N, T, D, H, DH = 4, 2048, 1024, 16, 64
N_CORES = 8
G = 512            # per-core projection width (8 heads x 64)
HPC = 8            # heads per core
SCALE = 1.0 / 8.0  # 1/sqrt(DH)

f32 = mybir.dt.float32
bf16 = mybir.dt.bfloat16
i32 = mybir.dt.int32

COMPUTE_DT = "bf16"  # operands are bf16 (host-converted); kept for test.py

# global din-block order produced by the two half-AllGathers:
# half 0 carries heads 0-3 (blocks 0,1) + peer heads 8-11 (blocks 4,5)
CC_PERM = [[0, 1, 4, 5], [2, 3, 6, 7]]


def _chunks(total, width):
    """[(offset, width)] covering [0, total) in pieces of at most width."""
    out = []
    t0 = 0
    while t0 < total:
        w = min(width, total - t0)
        out.append((t0, w))
        t0 += w
    return out


def build_nc(compute_dt: str = COMPUTE_DT, single_core: bool = False,
             reps: int = 0, u: int = T) -> bacc.Bacc:
    assert u % 128 == 0 and 128 <= u <= T

    nc = bacc.Bacc(
        "TRN2", target_bir_lowering=False, debug=False, num_devices=N_CORES
    )
    xT_d = nc.dram_tensor("xT", [D, T], bf16, kind="ExternalInput").ap()
    m_d = nc.dram_tensor("mask", [T], i32, kind="ExternalInput").ap()
    wqT_d = nc.dram_tensor("wqT", [D, G], bf16, kind="ExternalInput").ap()
    wkT_d = nc.dram_tensor("wkT", [D, G], bf16, kind="ExternalInput").ap()
    wvT_d = nc.dram_tensor("wvT", [D, G], bf16, kind="ExternalInput").ap()
    woT_d = nc.dram_tensor("woT", [D, G], bf16, kind="ExternalInput").ap()
    bq_d = nc.dram_tensor("bq", [G], f32, kind="ExternalInput").ap()
    bk_d = nc.dram_tensor("bk", [G], f32, kind="ExternalInput").ap()
    bv_d = nc.dram_tensor("bv", [G], f32, kind="ExternalInput").ap()
    bo_d = nc.dram_tensor("bo", [G], f32, kind="ExternalInput").ap()
    out_d = nc.dram_tensor("out", [u, G], f32, kind="ExternalOutput").ap()

    TB = T // 128   # 16 key token blocks
    DB = D // 128   # 8 feature blocks
    GB = G // 128   # 4 projected blocks
    UB = u // 128   # query token blocks
    qchunks = _chunks(u, 512)
    nch = len(qchunks)

    with tile.TileContext(nc) as tc, ExitStack() as outer_ctx:
        if reps:
            outer_ctx.enter_context(tc.For_i(0, reps, 1))
        ctx = outer_ctx.enter_context(ExitStack())
        const = ctx.enter_context(tc.tile_pool(name="const", bufs=1))
        bqk = const.tile([128, 2 * GB], f32, tag="bqk")
        bq_c, bk_c = bqk[:, 0:GB], bqk[:, GB:2 * GB]
        bvo = const.tile([128, 2 * G], f32, tag="bvo")
        bvb, bob = bvo[:, 0:G], bvo[:, G:2 * G]

        qpool = ctx.enter_context(tc.tile_pool(name="qpool", bufs=1))
        q_t = [qpool.tile([128, u], bf16, tag=f"q{i}", name=f"q{i}")
               for i in range(GB)]
        k_t = [qpool.tile([128, T], bf16, tag=f"k{i}", name=f"k{i}")
               for i in range(GB)]
        v_all = qpool.tile([128, TB * HPC * 65], bf16, tag="v_all")
        v_t = [v_all[:, i * HPC * 65:(i + 1) * HPC * 65] for i in range(TB)]

        dram = ctx.enter_context(tc.tile_pool(name="dram", bufs=1, space="DRAM"))
        cc_in = dram.tile([G, u], bf16)
        cc_out = [dram.tile([G, u], bf16, name=f"cc_out{c}", tag=f"cc_out{c}")
                  for c in range(2)]

        mkpool = ctx.enter_context(tc.tile_pool(name="maskp", bufs=1))
        maskb = mkpool.tile([128, u], f32, tag="maskb")

        # ---- mask + biases ----
        with tc.tile_pool(name="mload", bufs=1) as mp:
            m_i = mp.tile([1, T], i32)
            nc.sync.dma_start(m_i[:], m_d[None, :])
            # broadcast the i32 row first, THEN cast on 128 lanes — an
            # [1, u] single-lane cast costs ~3 cyc/elem on one DVE lane
            m_b = mp.tile([128, u], i32, tag="mbi")
            nc.gpsimd.partition_broadcast(m_b[:], m_i[:, 0:u])
            nc.vector.tensor_copy(maskb[:], m_b[:])
            nc.sync.dma_start(bq_c[:], bq_d.rearrange("(j p) -> p j", p=128))
            nc.sync.dma_start(bk_c[:], bk_d.rearrange("(j p) -> p j", p=128))
            bv_r = mp.tile([1, G], f32, tag="bvr")
            nc.sync.dma_start(bv_r[:], bv_d[None, :])
            nc.gpsimd.partition_broadcast(bvb[:], bv_r[:])
            bo_r = mp.tile([1, G], f32, tag="bor")
            nc.sync.dma_start(bo_r[:], bo_d[None, :])
            nc.gpsimd.partition_broadcast(bob[:], bo_r[:])

        # attention pools open FIRST so phase-1 pools can release while
        # attention is running (pool stack releases LIFO)
        att = ctx.enter_context(ExitStack())
        slabp = att.enter_context(tc.tile_pool(name="slab", bufs=2))
        zp = att.enter_context(tc.tile_pool(name="zbuf", bufs=2))
        csp = att.enter_context(tc.tile_pool(name="cstage", bufs=3))
        spp = att.enter_context(tc.tile_pool(name="spsum", bufs=2,
                                             space="PSUM"))

        # ---- phase 1: direct loads + K/Q projections ----
        p1 = ctx.enter_context(ExitStack())
        xtp = p1.enter_context(tc.tile_pool(name="xt", bufs=1))
        wtp = p1.enter_context(tc.tile_pool(name="wt", bufs=24))
        pp = p1.enter_context(tc.tile_pool(name="pp", bufs=4, space="PSUM"))

        xt = [xtp.tile([128, T], bf16, tag=f"xt{d}", name=f"xt{d}")
              for d in range(DB)]
        for tch in range(4):
            for d in range(DB):
                nc.sync.dma_start(
                    xt[d][:, tch * 512:(tch + 1) * 512],
                    xT_d[d * 128:(d + 1) * 128, tch * 512:(tch + 1) * 512],
                )

        def load_wT(w_dram):
            tiles = [wtp.tile([128, G], bf16, tag="wt", name="wt")
                     for _ in range(DB)]
            for d in range(DB):
                nc.sync.dma_start(tiles[d][:],
                                  w_dram[d * 128:(d + 1) * 128, :])
            return tiles

        wkT_t = load_wT(wkT_d)
        wqT_t = load_wT(wqT_d)
        wvT_t = load_wT(wvT_d)

        # K^T feature-major [dout, token]
        for b in range(GB):
            for tch in range(4):
                ps = pp.tile([128, 512], f32, tag="pp", name="pp")
                for d in range(DB):
                    nc.tensor.matmul(
                        ps[:],
                        wkT_t[d][:, b * 128:(b + 1) * 128],
                        xt[d][:, tch * 512:(tch + 1) * 512],
                        start=(d == 0),
                        stop=(d == DB - 1),
                    )
                nc.vector.tensor_scalar_add(
                    k_t[b][:, tch * 512:(tch + 1) * 512], ps[:],
                    bk_c[:, b:b + 1]
                )
        # Q^T with masked-query columns zeroed (-> uniform softmax rows,
        # matching the reference's -1e20 masked_fill exactly)
        for b in range(GB):
            for (t0, w) in qchunks:
                ps = pp.tile([128, 512], f32, tag="pp", name="pp")
                for d in range(DB):
                    nc.tensor.matmul(
                        ps[:, 0:w],
                        wqT_t[d][:, b * 128:(b + 1) * 128],
                        xt[d][:, t0:t0 + w],
                        start=(d == 0),
                        stop=(d == DB - 1),
                    )
                nc.vector.scalar_tensor_tensor(
                    q_t[b][:, t0:t0 + w],
                    ps[:, 0:w],
                    bq_c[:, b:b + 1],
                    maskb[:, t0:t0 + w],
                    op0=mybir.AluOpType.add,
                    op1=mybir.AluOpType.mult,
                )

        def emit_v_proj():
            # V token-major [t, dout] with a ones column per head
            nc.gpsimd.memset(v_all[:], 1.0)
            for i in range(TB):
                ps = pp.tile([128, 512], f32, tag="pp", name="pp")
                for d in range(DB):
                    nc.tensor.matmul(
                        ps[:],
                        xt[d][:, i * 128:(i + 1) * 128],
                        wvT_t[d][:],
                        start=(d == 0),
                        stop=(d == DB - 1),
                    )
                for h in range(HPC):
                    nc.vector.tensor_tensor(
                        v_t[i][:, h * 65:h * 65 + 64],
                        ps[:, h * 64:(h + 1) * 64],
                        bvb[:, h * 64:(h + 1) * 64],
                        op=mybir.AluOpType.add,
                    )

        # ---- phase 2: attention per head pair; ctx one unit behind ----
        # full 512-wide chunks are per-pair units; a 128-wide tail chunk is
        # merged across all 4 pairs into ONE unit so its exp stays N=1024
        full_chunks = [(t0, w) for (t0, w) in qchunks if w == 512]
        tail_chunks = [(t0, w) for (t0, w) in qchunks if w != 512]
        merged_tail = len(tail_chunks) == 1 and tail_chunks[0][1] == 128
        full_units = [("pair", p, t0, w) for p in range(GB)
                      for (t0, w) in full_chunks]
        if merged_tail:
            tail_units = [("tail", None, *tail_chunks[0])]
        else:
            tail_units = [("pair", p, t0, w) for p in range(GB)
                          for (t0, w) in tail_chunks]
        if full_units:
            units = [full_units[0]] + tail_units + full_units[1:]
        else:
            units = tail_units
        # AllGather half 0 needs pairs 0/1 (cc_in rows 0:256) complete
        gather0_idx = max(
            i for i, un in enumerate(units)
            if un[0] == "tail" or un[1] in (0, 1)
        ) + 1
        slabs = {}

        def recip_row(zsb, n):
            # reciprocal of a [1, n] row: a single-lane DVE reciprocal runs
            # ~6 cyc/elem on ONE lane (5-6us for n=1024), so scatter the row
            # across 128 partitions, recip there, and gather it back
            if n % 128 == 0:
                k = n // 128
                zs = zp.tile([128, 8], f32, tag="zs", name="zs")
                nc.sync.dma_start(zs[:, 0:k], zsb[0:1, 0:n])
                nc.vector.reciprocal(zs[:, 0:k], zs[:, 0:k])
                nc.sync.dma_start(zsb[0:1, 0:n], zs[:, 0:k])
            else:
                nc.vector.reciprocal(zsb[0:1, 0:n], zsb[0:1, 0:n])

        def emit_s_exp(unit):
            # the two heads' K=64 matmuls run CONCURRENTLY (row groups 0/64),
            # so head B always gets its own PSUM bank (offset 512) — two
            # concurrent drains into one bank are a hard PSUM fault
            _, p, t0, w = unit
            slab = slabp.tile([128, TB * 1024], bf16, tag="slab", name="slab")
            slabs[unit] = slab
            for j in range(TB):
                sps = spp.tile([128, 1024], f32, tag="sp", name="sp")
                nc.tensor.matmul(
                    sps[:, 0:w],
                    k_t[p][0:64, j * 128:(j + 1) * 128],
                    q_t[p][0:64, t0:t0 + w],
                    start=True, stop=True,
                )
                nc.tensor.matmul(
                    sps[:, 512:512 + w],
                    k_t[p][64:128, j * 128:(j + 1) * 128],
                    q_t[p][64:128, t0:t0 + w],
                    start=True, stop=True,
                )
                if w == 512:
                    nc.scalar.activation(
                        slab[:, j * 1024:j * 1024 + 1024],
                        sps[:],
                        mybir.ActivationFunctionType.Exp,
                        scale=SCALE,
                    )
                else:
                    for hh in range(2):
                        nc.scalar.activation(
                            slab[:, j * 1024 + hh * 512:
                                  j * 1024 + hh * 512 + w],
                            sps[:, hh * 512:hh * 512 + w],
                            mybir.ActivationFunctionType.Exp,
                            scale=SCALE,
                        )

        def emit_tail_s_exp(unit):
            # merged 128-wide tail: all 4 pairs in one PSUM layout, one
            # N=1024 exp per j.  A-heads (row group 0) fill bank pair 1,
            # B-heads (row group 64) fill bank pair 2.
            _, _, t0, w = unit
            slab = slabp.tile([128, TB * 1024], bf16, tag="slab", name="slab")
            slabs[unit] = slab
            for j in range(TB):
                sps = spp.tile([128, 1024], f32, tag="sp", name="sp")
                for p in range(GB):
                    nc.tensor.matmul(
                        sps[:, p * 128:p * 128 + w],
                        k_t[p][0:64, j * 128:(j + 1) * 128],
                        q_t[p][0:64, t0:t0 + w],
                        start=True, stop=True,
                    )
                    nc.tensor.matmul(
                        sps[:, 512 + p * 128:512 + p * 128 + w],
                        k_t[p][64:128, j * 128:(j + 1) * 128],
                        q_t[p][64:128, t0:t0 + w],
                        start=True, stop=True,
                    )
                nc.scalar.activation(
                    slab[:, j * 1024:(j + 1) * 1024],
                    sps[:],
                    mybir.ActivationFunctionType.Exp,
                    scale=SCALE,
                )

        late = ExitStack()
        woT = []
        cf0 = []
        cpp = None

        def open_late_pools():
            # called after p1.close() so cpsum can take the freed PSUM banks
            nonlocal cpp
            cpp = late.enter_context(
                tc.tile_pool(name="cpsum", bufs=4, space="PSUM"))
            woTp = late.enter_context(tc.tile_pool(name="wot", bufs=8))
            cf0p = late.enter_context(tc.tile_pool(name="cf0", bufs=1))
            woT.extend(woTp.tile([128, G], bf16, tag="wot", name="wot")
                       for _ in range(DB))
            for d in range(DB):
                nc.sync.dma_start(woT[d][:], woT_d[d * 128:(d + 1) * 128, :])
            cf0.extend(cf0p.tile([128, u], bf16, tag=f"cf0{j}",
                                 name=f"cf0{j}")
                       for j in range(GB))

        def emit_collective(c, cf_tiles):
            if single_core:
                nc.sync.dma_start(
                    cc_out[c][0:256, :], cc_in[c * 256:(c + 1) * 256, :]
                )
                nc.sync.dma_start(
                    cc_out[c][256:512, :], cc_in[c * 256:(c + 1) * 256, :]
                )
            else:
                nc.gpsimd.collective_compute(
                    "AllGather",
                    mybir.AluOpType.bypass,
                    replica_groups=[[0, 1], [2, 3], [4, 5], [6, 7]],
                    ins=[cc_in[c * 256:(c + 1) * 256, :].opt()],
                    outs=[cc_out[c][:].opt()],
                )
            for j in range(GB):
                nc.sync.dma_start(
                    cf_tiles[j][:], cc_out[c][j * 128:(j + 1) * 128, :]
                )

        def emit_post(unit):
            _, p, t0, w = unit
            b_off = 512
            slab = slabs.pop(unit)
            # ctx^T (+Z row) per head of the pair
            cps = []
            for hh in range(2):
                cp = cpp.tile([65, 512], f32, tag="cp", name="cp")
                cps.append(cp)
                for j in range(TB):
                    nc.tensor.matmul(
                        cp[:, 0:w],
                        v_t[j][:, (2 * p + hh) * 65:(2 * p + hh) * 65 + 65],
                        slab[:, j * 1024 + hh * b_off:
                              j * 1024 + hh * b_off + w],
                        start=(j == 0),
                        stop=(j == TB - 1),
                    )
            # row 64 holds Z = sum_k exp; scale rows 0..63 by 1/Z
            zsb = zp.tile([128, 1024], f32, tag="z", name="z")
            nc.vector.tensor_copy(zsb[64:65, 0:w], cps[0][64:65, 0:w])
            nc.vector.tensor_copy(zsb[64:65, w:2 * w], cps[1][64:65, 0:w])
            nc.sync.dma_start(zsb[0:1, 0:2 * w], zsb[64:65, 0:2 * w])
            recip_row(zsb, 2 * w)
            bct = zp.tile([64, 1024], f32, tag="bc", name="bc")
            nc.gpsimd.partition_broadcast(bct[:, 0:2 * w], zsb[0:1, 0:2 * w])
            for hh in range(2):
                cst = csp.tile([64, 512], bf16, tag="cst", name="cst")
                nc.vector.tensor_tensor(
                    cst[:, 0:w], cps[hh][0:64, 0:w],
                    bct[:, hh * w:(hh + 1) * w],
                    op=mybir.AluOpType.mult,
                )
                nc.sync.dma_start(
                    cc_in[p * 128 + hh * 64:p * 128 + hh * 64 + 64,
                          t0:t0 + w],
                    cst[:, 0:w],
                )

        def emit_tail_post(unit):
            _, _, t0, w = unit
            slab = slabs.pop(unit)
            # ctx for all 8 heads: A-heads (even) share one PSUM bank in
            # 128-col lanes, B-heads (odd) another — ctx matmuls contract
            # over the full 128 rows so they execute serially (no
            # concurrent same-bank drains)
            cps = []
            for hh in range(2):
                cp = cpp.tile([65, 512], f32, tag="cp", name="cp")
                cps.append(cp)
                for p in range(GB):
                    for j in range(TB):
                        nc.tensor.matmul(
                            cp[:, p * 128:p * 128 + w],
                            v_t[j][:, (2 * p + hh) * 65:
                                  (2 * p + hh) * 65 + 65],
                            slab[:, j * 1024 + hh * 512 + p * 128:
                                  j * 1024 + hh * 512 + p * 128 + w],
                            start=(j == 0),
                            stop=(j == TB - 1),
                        )
            zsb = zp.tile([128, 1024], f32, tag="z", name="z")
            nc.vector.tensor_copy(zsb[64:65, 0:512], cps[0][64:65, 0:512])
            nc.vector.tensor_copy(zsb[64:65, 512:1024], cps[1][64:65, 0:512])
            nc.sync.dma_start(zsb[0:1, :], zsb[64:65, :])
            recip_row(zsb, 1024)
            bct = zp.tile([64, 1024], f32, tag="bc", name="bc")
            nc.gpsimd.partition_broadcast(bct[:], zsb[0:1, :])
            for hh in range(2):
                cst = csp.tile([64, 512], bf16, tag="cst", name="cst")
                nc.vector.tensor_tensor(
                    cst[:], cps[hh][0:64, :], bct[:, hh * 512:(hh + 1) * 512],
                    op=mybir.AluOpType.mult,
                )
                for p in range(GB):
                    nc.sync.dma_start(
                        cc_in[p * 128 + hh * 64:p * 128 + hh * 64 + 64,
                              t0:t0 + w],
                        cst[:, p * 128:p * 128 + w],
                    )

        def emit_unit(unit):
            if unit[0] == "tail":
                emit_tail_s_exp(unit)
            else:
                emit_s_exp(unit)

        def emit_unit_post(unit):
            if unit[0] == "tail":
                emit_tail_post(unit)
            else:
                emit_post(unit)

        for idx, unit in enumerate(units):
            emit_unit(unit)
            if idx == 0:
                # V projection runs under the first unit's exp window
                emit_v_proj()
                p1.close()
                open_late_pools()
            else:
                emit_unit_post(units[idx - 1])
            if idx == gather0_idx:
                emit_collective(0, cf0)
        emit_unit_post(units[-1])
        if gather0_idx >= len(units):
            emit_collective(0, cf0)

        # ---- phase 3: second gather half + output projection ----
        # cf1 reuses a slab slot (attention has drained by now)
        cf1_all = slabp.tile([128, TB * 1024], bf16, tag="slab", name="cf1")
        cf1 = [cf1_all[:, j * u:(j + 1) * u] for j in range(GB)]
        emit_collective(1, cf1)
        cf = [cf0, cf1]
        for i in range(UB):
            ps = cpp.tile([128, 512], f32, tag="cp", name="cp")
            first = True
            for c in range(2):
                for j in range(GB):
                    nc.tensor.matmul(
                        ps[:],
                        cf[c][j][:, i * 128:(i + 1) * 128],
                        woT[CC_PERM[c][j]][:],
                        start=first,
                        stop=(c == 1 and j == GB - 1),
                    )
                    first = False
            os_ = csp.tile([128, G], f32, tag="ostage", name="ostage")
            nc.vector.tensor_tensor(os_[:], ps[:], bob[:],
                                    op=mybir.AluOpType.add)
            nc.sync.dma_start(out_d[i * 128:(i + 1) * 128, :], os_[:])
        late.close()

    nc.compile()
    return nc


def _prep_permutation(mask):
    """Per-batch permutation putting unmasked queries first, and the padded
    processed-query count U (multiple of 128, > U_n for every batch with
    masked queries)."""
    perms, uns = [], []
    for n in range(N):
        m = np.asarray(mask[n])
        unm = np.flatnonzero(m != 0)
        msk = np.flatnonzero(m == 0)
        perms.append(np.concatenate([unm, msk]).astype(np.int64))
        uns.append(int(len(unm)))
    umax = max(uns)
    u = min(T, ((umax + 1 + 127) // 128) * 128)
    return perms, uns, u


def shard_inputs(query, mask, Wq, bq, Wk, bk, Wv, bv, Wo, bo):
    perms, uns, u = _prep_permutation(mask)
    meta = {"perms": perms, "uns": uns, "u": u}
    nbf = mybir.dt.np(bf16)
    in_maps = []
    for c in range(N_CORES):
        n, g = c // 2, c % 2
        sl = slice(g * G, (g + 1) * G)
        in_maps.append(
            {
                "xT": np.ascontiguousarray(
                    np.asarray(query[n])[perms[n]].T.astype(nbf)),
                "mask": np.ascontiguousarray(
                    np.asarray(mask[n])[perms[n]], dtype=np.int32),
                "wqT": np.ascontiguousarray(Wq[sl].T.astype(nbf)),
                "wkT": np.ascontiguousarray(Wk[sl].T.astype(nbf)),
                "wvT": np.ascontiguousarray(Wv[sl].T.astype(nbf)),
                "woT": np.ascontiguousarray(Wo[sl].T.astype(nbf)),
                "bq": np.ascontiguousarray(bq[sl], dtype=np.float32),
                "bk": np.ascontiguousarray(bk[sl], dtype=np.float32),
                "bv": np.ascontiguousarray(bv[sl], dtype=np.float32),
                "bo": np.ascontiguousarray(bo[sl], dtype=np.float32),
            }
        )
    return in_maps, meta


def gather_outputs(results, meta):
    perms, uns, u = meta["perms"], meta["uns"], meta["u"]
    out = np.empty((N, T, D), np.float32)
    for n in range(N):
        dev = np.empty((u, D), np.float32)
        for g in range(2):
            dev[:, g * G:(g + 1) * G] = results[2 * n + g]["out"]
        perm = perms[n]
        out[n][perm[:u]] = dev
        if u < T:
            # all masked queries share the uniform-attention output row;
            # uns[n] < u is guaranteed, and row uns[n] is masked
            out[n][perm[u:]] = dev[uns[n]]
    return out


def kernel(query, mask, Wq, bq, Wk, bk, Wv, bv, Wo, bo):
    in_maps, meta = shard_inputs(query, mask, Wq, bq, Wk, bk, Wv, bv, Wo, bo)
    nc = build_nc(u=meta["u"])
    res = run_bass_kernel_spmd(nc, in_maps, list(range(N_CORES)))
    return gather_outputs(res.results, meta)
